# revision 2
# baseline (speedup 1.0000x reference)
"""Distributed exact inner-product top-k (brute-force kNN) on 8 TRN2 NeuronCores.

Sharding: codebook W is split row-wise into 8 shards of 25000 (one per core);
x is replicated.  Host pre-transposes both into fp8e4 with the contraction
dim (128) split as two 64-dim k-tiles so the PE can run fp8 DoubleRow
matmuls (2 output columns per cycle instead of 1).

Device kernel (SPMD, identical graph per core, no collectives):
  - per 2048-wide vocab region (4 PSUM banks, 2 in flight): 4x fp8 DoubleRow
    matmuls [64x2 contraction, 512 cols] into PSUM (f32)
  - each region's scores are drained by one of the only two engines that can
    read PSUM, statically balanced by their clocks (DVE 0.96 GHz, Act 1.2):
      D: DVE windowed tensor_reduce(max) w=4 -> bf16 window maxima
      A: Act copy PSUM -> fp8e4 raw scores (window-1)
    one region per group is split between the two so the ratio lands at the
    balance point (DVE 11524 cols : Act 13476 cols per 25000)
  - per-row outputs: 2881 bf16 w4-maxima + 13476 fp8 raw scores, DMA'd in
    region-granular chunks alternating between the SP and gpsimd DMA queues

Host merge (the all-gather + final top-k of the distributed ANN pattern):
  - per row, select every window whose (value + its route's EPS) clears
    (128th-largest window value - EPSMAX - slack); gather member columns
  - exact f64 re-rank of the candidates; final top-128 ordered like
    jax.lax.top_k (value desc, index asc)
  - exactness guard: containment holds if |device value - exact window max|
    <= EPS_route for every window; EPS is validated per-run on all selected
    windows and violating rows (expected none) are recomputed exactly.
"""

import numpy as np

B = 1024
D = 128
VOCAB = 200000
NCORES = 8
VSHARD = VOCAB // NCORES  # 25000
REG = 2048  # 4 PSUM banks of f32
NREG = 12  # full regions per shard
TAIL = VSHARD - NREG * REG  # 424
TOPK = 128

# Per-group region schedule (same for every group).  Route "D" = DVE w4
# reduce, "A" = Act fp8 copy, "S" = split (first SPLIT_D cols to DVE, rest
# to Act).  Interleaved so both drain engines stream continuously.
ROUTES = ["D", "A", "D", "A", "D", "A", "D", "A", "D", "A", "S", "A"]
SPLIT_D = 860  # multiple of 4; balances 0.96GHz DVE vs 1.2GHz Act

# Segment tables: (shard_col_lo, n_cols) in write order.
W4_SEGS = []
W1_SEGS = []
for _r, _route in enumerate(ROUTES):
    _lo = _r * REG
    if _route == "D":
        W4_SEGS.append((_lo, REG))
    elif _route == "A":
        W1_SEGS.append((_lo, REG))
    else:
        W4_SEGS.append((_lo, SPLIT_D))
        W1_SEGS.append((_lo + SPLIT_D, REG - SPLIT_D))
W4_SEGS.append((NREG * REG, TAIL))  # tail region -> DVE
NW4 = sum(n // 4 for _, n in W4_SEGS)  # 2881
NW1 = sum(n for _, n in W1_SEGS)  # 13476
W4_OFF = np.concatenate([[0], np.cumsum([n // 4 for _, n in W4_SEGS])])
W1_OFF = np.concatenate([[0], np.cumsum([n for _, n in W1_SEGS])])

# |device window value - exact window max| bounds, validated at runtime:
# fp8e4 input quantization noise on both x and W (std ~0.6, observed max
# ~3) plus output quantization (bf16 ~0.2 for route D, fp8e4 ~2.8 for
# route A at |s|~45).
EPS4 = 4.25
EPS1 = 7.0
EPSMAX = EPS1
SLACK = 0.5

LAST_RESULTS = None  # BassKernelResults of the most recent run (for profiling)
_CACHED_NC = None


def build_kernel():
    import concourse.bass as bass  # noqa: F401
    import concourse.tile as tile
    from concourse import bacc, mybir

    F32 = mybir.dt.float32
    BF16 = mybir.dt.bfloat16
    FP8 = mybir.dt.float8e4
    AX = mybir.AxisListType.X
    MAX = mybir.AluOpType.max
    COPY = mybir.ActivationFunctionType.Copy
    DR = mybir.MatmulPerfMode.DoubleRow

    nc = bacc.Bacc("TRN2", target_bir_lowering=False, debug=False)
    wt_d = nc.dram_tensor("wt", [64, 2, VSHARD], FP8, kind="ExternalInput")
    xt_d = nc.dram_tensor("xt", [64, 2, B], FP8, kind="ExternalInput")
    out4_d = nc.dram_tensor("out_w4", [B, NW4], BF16, kind="ExternalOutput")
    out1_d = nc.dram_tensor("out_w1", [B, NW1], FP8, kind="ExternalOutput")

    with tile.TileContext(nc) as tc:
        with (
            tc.tile_pool(name="wt", bufs=1) as wt_pool,
            tc.tile_pool(name="xt", bufs=1) as xt_pool,
            tc.tile_pool(name="psum", bufs=2, space="PSUM") as psum_pool,
            tc.tile_pool(name="outw", bufs=2) as out_pool,
        ):
            wt_sb = wt_pool.tile([64, 2, VSHARD], FP8)
            xt_sb = xt_pool.tile([64, 2, B], FP8)
            # xt first: the first matmul's stationary operand should not wait
            # behind the whole 3.2MB W load.  W is split into slabs in
            # consumption order, alternating between the SP HW queue and the
            # gpsimd software-DGE queue.
            nc.sync.dma_start(xt_sb[:], xt_d[:])
            slabs = [512] * 8
            rest = VSHARD - sum(slabs)
            nrest = 24
            for s in range(nrest):
                slabs.append(rest // nrest + (1 if s < rest % nrest else 0))
            lo = 0
            for s, w in enumerate(slabs):
                eng = nc.sync if s % 2 == 0 else nc.gpsimd
                eng.dma_start(wt_sb[:, :, lo:lo + w], wt_d[:, :, lo:lo + w])
                lo += w
            assert lo == VSHARD

            # DMA-out cuts: (after_region_index, which, win_lo, win_hi)
            cuts = [
                (5, 1, 0, int(W1_OFF[3])),
                (6, 4, 0, int(W4_OFF[4])),
                (9, 1, int(W1_OFF[3]), int(W1_OFF[5])),
                (11, 1, int(W1_OFF[5]), NW1),
                (12, 4, int(W4_OFF[4]), NW4),
            ]

            for g in range(B // 128):
                out4_sb = out_pool.tile([128, NW4], BF16, tag="out4")
                out1_sb = out_pool.tile([128, NW1], FP8, tag="out1")
                xg = xt_sb[:, :, g * 128:(g + 1) * 128]
                i4 = 0
                i1 = 0
                for r in range(NREG + 1):
                    base = r * REG
                    w_cols = REG if r < NREG else TAIL
                    route = ROUTES[r] if r < NREG else "D"
                    ps = psum_pool.tile([128, REG], F32)
                    for k in range(0, w_cols, 512):
                        kw = min(512, w_cols - k)
                        nc.tensor.matmul(
                            ps[:, k:k + kw],
                            xg,
                            wt_sb[:, :, base + k:base + k + kw],
                            start=True, stop=True,
                            perf_mode=DR,
                        )
                    if route in ("D", "S"):
                        nd = w_cols if route == "D" else SPLIT_D
                        o4 = int(W4_OFF[i4])
                        nc.vector.tensor_reduce(
                            out4_sb[:, o4:o4 + nd // 4],
                            ps[:, :nd].rearrange("p (n w) -> p n w", w=4),
                            axis=AX, op=MAX,
                        )
                        i4 += 1
                    if route in ("A", "S"):
                        lo1 = 0 if route == "A" else SPLIT_D
                        o1 = int(W1_OFF[i1])
                        nc.scalar.activation(
                            out1_sb[:, o1:o1 + w_cols - lo1],
                            ps[:, lo1:w_cols],
                            COPY,
                        )
                        i1 += 1
                    for ci, (cr, which, wlo, whi) in enumerate(cuts):
                        if cr != r:
                            continue
                        eng = nc.sync if (g + ci) % 2 == 0 else nc.gpsimd
                        src = out4_sb if which == 4 else out1_sb
                        dst = out4_d if which == 4 else out1_d
                        eng.dma_start(
                            dst[g * 128:(g + 1) * 128, wlo:whi],
                            src[:, wlo:whi],
                        )
    nc.compile()
    return nc


def _build_maps():
    """Per-window candidate columns and EPS.

    Returns (colmap [NWIN, 4] int64 with -1 pads, eps [NWIN] f32) where
    window order is [all w4 windows, all w1 windows] per core.
    """
    nwin = NW4 + NW1
    cm = np.full((nwin, 4), -1, np.int64)
    eps = np.empty(nwin, np.float32)
    for si, (lo, n) in enumerate(W4_SEGS):
        o = int(W4_OFF[si])
        j = np.arange(n // 4)[:, None]
        cm[o:o + n // 4] = lo + 4 * j + np.arange(4)[None, :]
    eps[:NW4] = EPS4
    for si, (lo, n) in enumerate(W1_SEGS):
        o = NW4 + int(W1_OFF[si])
        cm[o:o + n, 0] = lo + np.arange(n)
    eps[NW4:] = EPS1
    return cm, eps


_COLMAP, _WEPS = _build_maps()


def _topk_rows(vals, gidx, k):
    """Per-row top-k ordered like jax.lax.top_k: value desc, index asc."""
    order = np.lexsort((gidx, -vals), axis=-1)[:, :k]
    return (
        np.take_along_axis(gidx, order, axis=1),
        np.take_along_axis(vals, order, axis=1),
    )


def kernel(x: np.ndarray, W: np.ndarray, topk) -> np.ndarray:
    global LAST_RESULTS, _CACHED_NC
    import os

    import ml_dtypes

    from concourse.bass_utils import run_bass_kernel_spmd

    assert x.shape == (B, D) and W.shape == (VOCAB, D)
    assert int(topk) == TOPK
    x = np.ascontiguousarray(np.asarray(x, dtype=np.float32))
    W = np.ascontiguousarray(np.asarray(W, dtype=np.float32))

    if _CACHED_NC is None:
        _CACHED_NC = build_kernel()
    nc = _CACHED_NC

    # [64, 2, B]: k-tile i holds dims [64i, 64i+64) of x^T
    xt = np.ascontiguousarray(
        x.T.reshape(2, 64, B).transpose(1, 0, 2)
    ).astype(ml_dtypes.float8_e4m3)
    in_maps = []
    for i in range(NCORES):
        wt_i = np.ascontiguousarray(
            W[i * VSHARD:(i + 1) * VSHARD].T.reshape(2, 64, VSHARD)
            .transpose(1, 0, 2)
        ).astype(ml_dtypes.float8_e4m3)
        in_maps.append({"wt": wt_i, "xt": xt})

    LAST_RESULTS = run_bass_kernel_spmd(
        nc,
        in_maps,
        core_ids=list(range(NCORES)),
        trace=bool(int(os.environ.get("KERNEL_TRACE", "0"))),
    )
    results = LAST_RESULTS.results

    # [B, 8*(NW4+NW1)] device window values, f32
    nwin = NW4 + NW1
    wm = np.empty((B, NCORES * nwin), np.float32)
    for i in range(NCORES):
        wm[:, i * nwin:i * nwin + NW4] = np.asarray(
            results[i]["out_w4"]).astype(np.float32)
        wm[:, i * nwin + NW4:(i + 1) * nwin] = np.asarray(
            results[i]["out_w1"]).astype(np.float32)
    nwin_all = NCORES * nwin
    weps_all = np.tile(_WEPS, NCORES)

    # Per-row selection on adjusted values v' = v + eps_w:
    # keep windows with v' >= kth_dev - EPSMAX - SLACK.
    wma = wm + weps_all[None, :]
    kth = np.partition(wm, nwin_all - TOPK, axis=1)[:, nwin_all - TOPK]
    tau = kth - EPSMAX - SLACK
    counts = (wma >= tau[:, None]).sum(axis=1)
    K = int(min(max(int(counts.max()), TOPK + 64), 8192))
    topw = np.argpartition(-wma, K - 1, axis=1)[:, :K]  # [B, K] window ids

    core_id = topw // nwin
    wi = topw % nwin
    cols = _COLMAP[wi]  # [B, K, 4], -1 = pad
    pad = cols < 0
    cand = (np.where(pad, 0, cols) + core_id[..., None] * VSHARD).reshape(B, K * 4)

    # Exact f64 re-rank of the candidate columns (pads scored -inf).
    x64 = x.astype(np.float64)
    W64 = W.astype(np.float64)
    exact = np.empty((B, K * 4), np.float64)
    STEP = 64
    for r0 in range(0, B, STEP):
        r1 = r0 + STEP
        gW = W64[cand[r0:r1]]  # [STEP, K*4, D]
        exact[r0:r1] = np.einsum("bjd,bd->bj", gW, x64[r0:r1])
    exact[pad.reshape(B, K * 4)] = -np.inf

    gidx_top, vals_top = _topk_rows(exact, cand, TOPK)

    # Exactness guards.
    t128 = vals_top[:, -1]
    dev_w = np.take_along_axis(wm, topw, axis=1)
    true_w = exact.reshape(B, K, 4).max(axis=2)
    werr = np.abs(dev_w - true_w)
    sel_eps = weps_all[topw]
    err_excess = (werr - sel_eps).max(axis=1)
    bad = (
        (err_excess > 0)
        | (tau + EPSMAX > t128)
        | (counts > K)
    )
    if os.environ.get("KERNEL_DEBUG"):
        w4mask = (topw % nwin) < NW4
        e4 = werr[w4mask].max() if w4mask.any() else 0.0
        e1 = werr[~w4mask].max() if (~w4mask).any() else 0.0
        print(f"[kernel] K={K} counts max={counts.max()} "
              f"err4 max={e4:.3f} err1 max={e1:.3f} bad rows={int(bad.sum())}")
    for r in np.flatnonzero(bad):
        s = x64[r] @ W64.T
        gidx_top[r] = np.lexsort((np.arange(VOCAB), -s))[:TOPK]

    return gidx_top.astype(np.int32)


# revision 4
# speedup vs baseline: 1.2181x; 1.2181x over previous
"""Distributed exact inner-product top-k (brute-force kNN) on 8 TRN2 NeuronCores.

Sharding: codebook W is split row-wise into 8 shards of 25000 (one per core);
x is replicated.  Host pre-transposes both to bf16: the PE streams 1 output
column/cycle regardless of dtype at contraction 128 (fp8 DoubleRow only
pays off at contraction 256 - measured), so bf16 is free accuracy.

Device kernel (SPMD, identical graph per core, no collectives):
  - per 2048-wide vocab region (4 PSUM banks, 2 in flight): 4x bf16
    matmuls [128 contraction, 512 cols] into PSUM (f32)
  - each region's scores are drained by one of the only two engines that can
    read PSUM, statically balanced by their clocks (DVE 0.96 GHz, Act 1.2):
      D: DVE windowed tensor_reduce(max) w=4 -> bf16 window maxima
      A: Act copy PSUM -> fp8e4 raw scores (window-1)
    one region per group is split between the two so the ratio lands at the
    balance point (DVE 11524 cols : Act 13476 cols per 25000)
  - per-row outputs: 2881 bf16 w4-maxima + 13476 fp8 raw scores, DMA'd in
    region-granular chunks alternating between the SP and gpsimd DMA queues

Host merge (the all-gather + final top-k of the distributed ANN pattern):
  - per row, select every window whose (value + its route's EPS) clears
    (128th-largest window value - EPSMAX - slack); gather member columns
  - exact f64 re-rank of the candidates; final top-128 ordered like
    jax.lax.top_k (value desc, index asc)
  - exactness guard: containment holds if |device value - exact window max|
    <= EPS_route for every window; EPS is validated per-run on all selected
    windows and violating rows (expected none) are recomputed exactly.
    bf16 inputs keep the device/exact gap tiny (bf16-out windows ~0.15,
    fp8e4-out windows ~2.2 at |s|~45), so margins and candidate counts stay
    small and the host merge is cheap.
"""

import numpy as np

B = 1024
D = 128
VOCAB = 200000
NCORES = 8
VSHARD = VOCAB // NCORES  # 25000
REG = 2048  # 4 PSUM banks of f32
NREG = 12  # full regions per shard
TAIL = VSHARD - NREG * REG  # 424
TOPK = 128

# Per-group region schedule (same for every group).  Route "D" = DVE w4
# reduce, "A" = Act fp8 copy, "S" = split (first SPLIT_D cols to DVE, rest
# to Act).  Interleaved so both drain engines stream continuously.
ROUTES = ["D", "A", "D", "A", "D", "A", "D", "A", "D", "A", "S", "A"]
SPLIT_D = 860  # multiple of 4; balances 0.96GHz DVE vs 1.2GHz Act

# Segment tables: (shard_col_lo, n_cols) in write order.
W4_SEGS = []
W1_SEGS = []
for _r, _route in enumerate(ROUTES):
    _lo = _r * REG
    if _route == "D":
        W4_SEGS.append((_lo, REG))
    elif _route == "A":
        W1_SEGS.append((_lo, REG))
    else:
        W4_SEGS.append((_lo, SPLIT_D))
        W1_SEGS.append((_lo + SPLIT_D, REG - SPLIT_D))
W4_SEGS.append((NREG * REG, TAIL))  # tail region -> DVE
NW4 = sum(n // 4 for _, n in W4_SEGS)  # 2881
NW1 = sum(n for _, n in W1_SEGS)  # 13476
W4_OFF = np.concatenate([[0], np.cumsum([n // 4 for _, n in W4_SEGS])])
W1_OFF = np.concatenate([[0], np.cumsum([n for _, n in W1_SEGS])])

# |device window value - exact window max| bounds, validated at runtime:
# bf16 input quantization noise on x and W (~0.1) plus output quantization
# (bf16 ~0.15 for route D, fp8e4 ~2.2 for route A at |s|~45).
EPS4 = 1.0
EPS1 = 3.2
EPSMAX = EPS1
SLACK = 0.3

LAST_RESULTS = None  # BassKernelResults of the most recent run (for profiling)
_CACHED_NC = None


def build_kernel():
    import concourse.bass as bass  # noqa: F401
    import concourse.tile as tile
    from concourse import bacc, mybir

    F32 = mybir.dt.float32
    BF16 = mybir.dt.bfloat16
    FP8 = mybir.dt.float8e4
    AX = mybir.AxisListType.X
    MAX = mybir.AluOpType.max
    COPY = mybir.ActivationFunctionType.Copy

    nc = bacc.Bacc("TRN2", target_bir_lowering=False, debug=False)
    wt_d = nc.dram_tensor("wt", [D, VSHARD], BF16, kind="ExternalInput")
    xt_d = nc.dram_tensor("xt", [D, B], BF16, kind="ExternalInput")
    out4_d = nc.dram_tensor("out_w4", [B, NW4], BF16, kind="ExternalOutput")
    out1_d = nc.dram_tensor("out_w1", [B, NW1], FP8, kind="ExternalOutput")

    with tile.TileContext(nc) as tc:
        with (
            tc.tile_pool(name="wt", bufs=1) as wt_pool,
            tc.tile_pool(name="xt", bufs=1) as xt_pool,
            tc.tile_pool(name="psum", bufs=2, space="PSUM") as psum_pool,
            tc.tile_pool(name="outw", bufs=2) as out_pool,
        ):
            wt_sb = wt_pool.tile([D, VSHARD], BF16)
            xt_sb = xt_pool.tile([D, B], BF16)
            # xt first: the first matmul's stationary operand should not wait
            # behind the whole 3.2MB W load.  W is split into slabs in
            # consumption order, alternating between the SP HW queue and the
            # gpsimd software-DGE queue.
            nc.sync.dma_start(xt_sb[:], xt_d[:])
            slabs = [512] * 8
            rest = VSHARD - sum(slabs)
            nrest = 24
            for s in range(nrest):
                slabs.append(rest // nrest + (1 if s < rest % nrest else 0))
            lo = 0
            for s, w in enumerate(slabs):
                eng = nc.sync if s % 2 == 0 else nc.gpsimd
                eng.dma_start(wt_sb[:, lo:lo + w], wt_d[:, lo:lo + w])
                lo += w
            assert lo == VSHARD

            # DMA-out cuts: (after_region_index, which, win_lo, win_hi)
            cuts = [
                (5, 1, 0, int(W1_OFF[3])),
                (6, 4, 0, int(W4_OFF[4])),
                (9, 1, int(W1_OFF[3]), int(W1_OFF[5])),
                (11, 1, int(W1_OFF[5]), NW1),
                (12, 4, int(W4_OFF[4]), NW4),
            ]

            for g in range(B // 128):
                out4_sb = out_pool.tile([128, NW4], BF16, tag="out4")
                out1_sb = out_pool.tile([128, NW1], FP8, tag="out1")
                xg = xt_sb[:, g * 128:(g + 1) * 128]
                i4 = 0
                i1 = 0
                for r in range(NREG + 1):
                    base = r * REG
                    w_cols = REG if r < NREG else TAIL
                    route = ROUTES[r] if r < NREG else "D"
                    ps = psum_pool.tile([128, REG], F32)
                    for k in range(0, w_cols, 512):
                        kw = min(512, w_cols - k)
                        nc.tensor.matmul(
                            ps[:, k:k + kw],
                            xg,
                            wt_sb[:, base + k:base + k + kw],
                            start=True, stop=True,
                        )
                    if route in ("D", "S"):
                        nd = w_cols if route == "D" else SPLIT_D
                        o4 = int(W4_OFF[i4])
                        nc.vector.tensor_reduce(
                            out4_sb[:, o4:o4 + nd // 4],
                            ps[:, :nd].rearrange("p (n w) -> p n w", w=4),
                            axis=AX, op=MAX,
                        )
                        i4 += 1
                    if route in ("A", "S"):
                        lo1 = 0 if route == "A" else SPLIT_D
                        o1 = int(W1_OFF[i1])
                        nc.scalar.activation(
                            out1_sb[:, o1:o1 + w_cols - lo1],
                            ps[:, lo1:w_cols],
                            COPY,
                        )
                        i1 += 1
                    for ci, (cr, which, wlo, whi) in enumerate(cuts):
                        if cr != r:
                            continue
                        eng = nc.sync if (g + ci) % 2 == 0 else nc.gpsimd
                        src = out4_sb if which == 4 else out1_sb
                        dst = out4_d if which == 4 else out1_d
                        eng.dma_start(
                            dst[g * 128:(g + 1) * 128, wlo:whi],
                            src[:, wlo:whi],
                        )
    nc.compile()
    return nc


def _build_maps():
    """Per-window candidate columns and EPS.

    Returns (colmap [NWIN, 4] int64 with -1 pads, eps [NWIN] f32) where
    window order is [all w4 windows, all w1 windows] per core.
    """
    nwin = NW4 + NW1
    cm = np.full((nwin, 4), -1, np.int64)
    eps = np.empty(nwin, np.float32)
    for si, (lo, n) in enumerate(W4_SEGS):
        o = int(W4_OFF[si])
        j = np.arange(n // 4)[:, None]
        cm[o:o + n // 4] = lo + 4 * j + np.arange(4)[None, :]
    eps[:NW4] = EPS4
    for si, (lo, n) in enumerate(W1_SEGS):
        o = NW4 + int(W1_OFF[si])
        cm[o:o + n, 0] = lo + np.arange(n)
    eps[NW4:] = EPS1
    return cm, eps


_COLMAP, _WEPS = _build_maps()


def _topk_rows(vals, gidx, k):
    """Per-row top-k ordered like jax.lax.top_k: value desc, index asc."""
    order = np.lexsort((gidx, -vals), axis=-1)[:, :k]
    return (
        np.take_along_axis(gidx, order, axis=1),
        np.take_along_axis(vals, order, axis=1),
    )


def kernel(x: np.ndarray, W: np.ndarray, topk) -> np.ndarray:
    global LAST_RESULTS, _CACHED_NC
    import os

    import ml_dtypes

    from concourse.bass_utils import run_bass_kernel_spmd

    assert x.shape == (B, D) and W.shape == (VOCAB, D)
    assert int(topk) == TOPK
    x = np.ascontiguousarray(np.asarray(x, dtype=np.float32))
    W = np.ascontiguousarray(np.asarray(W, dtype=np.float32))

    if _CACHED_NC is None:
        _CACHED_NC = build_kernel()
    nc = _CACHED_NC

    xt = np.ascontiguousarray(x.T).astype(ml_dtypes.bfloat16)
    in_maps = []
    for i in range(NCORES):
        wt_i = np.ascontiguousarray(
            W[i * VSHARD:(i + 1) * VSHARD].T
        ).astype(ml_dtypes.bfloat16)
        in_maps.append({"wt": wt_i, "xt": xt})

    LAST_RESULTS = run_bass_kernel_spmd(
        nc,
        in_maps,
        core_ids=list(range(NCORES)),
        trace=bool(int(os.environ.get("KERNEL_TRACE", "0"))),
    )
    results = LAST_RESULTS.results

    # [B, 8*(NW4+NW1)] device window values, f32
    nwin = NW4 + NW1
    wm = np.empty((B, NCORES * nwin), np.float32)
    for i in range(NCORES):
        wm[:, i * nwin:i * nwin + NW4] = np.asarray(
            results[i]["out_w4"]).astype(np.float32)
        wm[:, i * nwin + NW4:(i + 1) * nwin] = np.asarray(
            results[i]["out_w1"]).astype(np.float32)
    nwin_all = NCORES * nwin
    weps_all = np.tile(_WEPS, NCORES)

    # Per-row selection on adjusted values v' = v + eps_w:
    # keep windows with v' >= kth_dev - EPSMAX - SLACK.
    wma = wm + weps_all[None, :]
    kth = np.partition(wm, nwin_all - TOPK, axis=1)[:, nwin_all - TOPK]
    tau = kth - EPSMAX - SLACK
    counts = (wma >= tau[:, None]).sum(axis=1)
    K = int(min(max(int(counts.max()), TOPK + 64), 4096))
    topw = np.argpartition(-wma, K - 1, axis=1)[:, :K]  # [B, K] window ids

    core_id = topw // nwin
    wi = topw % nwin
    cols = _COLMAP[wi]  # [B, K, 4], -1 = pad
    pad = cols < 0
    cand = (np.where(pad, 0, cols) + core_id[..., None] * VSHARD).reshape(B, K * 4)

    # Exact f64 re-rank of the candidate columns (pads scored -inf).
    x64 = x.astype(np.float64)
    W64 = W.astype(np.float64)
    exact = np.empty((B, K * 4), np.float64)
    STEP = 64
    for r0 in range(0, B, STEP):
        r1 = r0 + STEP
        gW = W64[cand[r0:r1]]  # [STEP, K*4, D]
        exact[r0:r1] = np.einsum("bjd,bd->bj", gW, x64[r0:r1])
    exact[pad.reshape(B, K * 4)] = -np.inf

    gidx_top, vals_top = _topk_rows(exact, cand, TOPK)

    # Exactness guards.
    t128 = vals_top[:, -1]
    dev_w = np.take_along_axis(wm, topw, axis=1)
    true_w = exact.reshape(B, K, 4).max(axis=2)
    werr = np.abs(dev_w - true_w)
    sel_eps = weps_all[topw]
    err_excess = (werr - sel_eps).max(axis=1)
    bad = (
        (err_excess > 0)
        | (tau + EPSMAX > t128)
        | (counts > K)
    )
    if os.environ.get("KERNEL_DEBUG"):
        w4mask = (topw % nwin) < NW4
        e4 = werr[w4mask].max() if w4mask.any() else 0.0
        e1 = werr[~w4mask].max() if (~w4mask).any() else 0.0
        print(f"[kernel] K={K} counts max={counts.max()} "
              f"err4 max={e4:.3f} err1 max={e1:.3f} bad rows={int(bad.sum())}")
    for r in np.flatnonzero(bad):
        s = x64[r] @ W64.T
        gidx_top[r] = np.lexsort((np.arange(VOCAB), -s))[:TOPK]

    return gidx_top.astype(np.int32)


# revision 5
# speedup vs baseline: 1.7990x; 1.4769x over previous
"""Distributed exact inner-product top-k (brute-force kNN) on 8 TRN2 NeuronCores.

Sharding: codebook W is split row-wise into 8 shards of 25000 (one per core);
x is replicated.  Host pre-transposes both to bf16: the PE streams 1 output
column/cycle regardless of dtype at contraction 128 (fp8 DoubleRow only
pays off at contraction 256 - measured), so bf16 is free accuracy.

Device kernel (SPMD, identical graph per core, no collectives):
  - per 1024-wide vocab region (2 PSUM banks, 4 in flight so both drain
    engines always have a ready region): 2x bf16 matmuls
    [128 contraction, 512 cols] into PSUM (f32)
  - each region's scores are drained by one of the only two engines that can
    read PSUM, statically balanced by their clocks and overheads
    (DVE 0.96 GHz reduce, Act 1.2 GHz copy):
      D: DVE windowed tensor_reduce(max) w=4 -> bf16 window maxima
         (11 regions + the 424-col tail)
      A: Act copy PSUM -> fp8e4 raw scores, window-1 (13 regions)
  - per-row outputs: 2922 bf16 w4-maxima + 13312 fp8 raw scores, DMA'd in
    multi-region chunks alternating between the SP and gpsimd DMA queues

Host merge (the all-gather + final top-k of the distributed ANN pattern):
  - per row, select every window whose (value + its route's EPS) clears
    (128th-largest window value - EPSMAX - slack); gather member columns
  - exact f64 re-rank of the candidates; final top-128 ordered like
    jax.lax.top_k (value desc, index asc)
  - exactness guard: containment holds if |device value - exact window max|
    <= EPS_route for every window that can matter; all such windows are
    selected, EPS is validated on them per-run, and violating rows
    (expected none) are recomputed exactly.  bf16 inputs keep the gap tiny
    (bf16-out windows ~0.25, fp8e4-out windows ~2.2 at |s|~45), so margins
    and candidate counts stay small and the host merge is cheap.
"""

import numpy as np

B = 1024
D = 128
VOCAB = 200000
NCORES = 8
VSHARD = VOCAB // NCORES  # 25000
REG = 1024  # 2 PSUM banks of f32
NREG = 24  # full regions per shard
TAIL = VSHARD - NREG * REG  # 424
TOPK = 128

# Per-group region schedule (same for every group).  Route "D" = DVE w4
# reduce, "A" = Act fp8 copy.  Interleaved so both engines stream off the
# 4 rotating PSUM buffers; region 22 goes to Act so the ratio lands near
# the 0.96/1.2 GHz balance point (DVE 11688 cols : Act 13312 per 25000).
ROUTES = ["D" if (r % 2 == 0 and r != 22) else "A" for r in range(NREG)]
ROUTES.append("D")  # tail region -> DVE

# Segment tables: (shard_col_lo, n_cols) in region order.
W4_SEGS = []
W1_SEGS = []
for _r, _route in enumerate(ROUTES):
    _lo = _r * REG
    _n = REG if _r < NREG else TAIL
    (W4_SEGS if _route == "D" else W1_SEGS).append((_lo, _n))
NW4 = sum(n // 4 for _, n in W4_SEGS)  # 2922
NW1 = sum(n for _, n in W1_SEGS)  # 13312
W4_OFF = np.concatenate([[0], np.cumsum([n // 4 for _, n in W4_SEGS])])
W1_OFF = np.concatenate([[0], np.cumsum([n for _, n in W1_SEGS])])

# |device window value - exact window max| bounds, validated at runtime:
# bf16 input quantization noise on x and W (~0.1-0.2) plus output
# quantization (bf16 ~0.2 for route D, fp8e4 ~2.2 for route A at |s|~45).
EPS4 = 0.45
EPS1 = 2.5
EPSMAX = EPS1
SLACK = 0.3

LAST_RESULTS = None  # BassKernelResults of the most recent run (for profiling)
_CACHED_NC = None


def build_kernel():
    import concourse.bass as bass  # noqa: F401
    import concourse.tile as tile
    from concourse import bacc, mybir

    F32 = mybir.dt.float32
    BF16 = mybir.dt.bfloat16
    FP8 = mybir.dt.float8e4
    AX = mybir.AxisListType.X
    MAX = mybir.AluOpType.max
    COPY = mybir.ActivationFunctionType.Copy

    nc = bacc.Bacc("TRN2", target_bir_lowering=False, debug=False)
    wt_d = nc.dram_tensor("wt", [D, VSHARD], BF16, kind="ExternalInput")
    xt_d = nc.dram_tensor("xt", [D, B], BF16, kind="ExternalInput")
    out4_d = nc.dram_tensor("out_w4", [B, NW4], BF16, kind="ExternalOutput")
    out1_d = nc.dram_tensor("out_w1", [B, NW1], FP8, kind="ExternalOutput")

    # DMA-out cuts: (after_region_index, which_stream, seg_lo, seg_hi)
    # expressed in completed-segment counts per stream.
    W4_DONE = np.cumsum([1 if rt == "D" else 0 for rt in ROUTES])
    W1_DONE = np.cumsum([1 if rt == "A" else 0 for rt in ROUTES])
    cuts = []
    prev1 = 0
    for cr in (5, 11, 17, 23):
        hi = int(W1_OFF[W1_DONE[cr]])
        cuts.append((cr, 1, prev1, hi))
        prev1 = hi
    prev4 = 0
    for cr in (12, NREG):
        hi = int(W4_OFF[W4_DONE[cr]])
        cuts.append((cr, 4, prev4, hi))
        prev4 = hi

    with tile.TileContext(nc) as tc:
        with (
            tc.tile_pool(name="wt", bufs=1) as wt_pool,
            tc.tile_pool(name="xt", bufs=1) as xt_pool,
            tc.tile_pool(name="psum", bufs=4, space="PSUM") as psum_pool,
            tc.tile_pool(name="outw", bufs=4) as out_pool,
        ):
            wt_sb = wt_pool.tile([D, VSHARD], BF16)
            xt_sb = xt_pool.tile([D, B], BF16)
            # xt first: the first matmul's stationary operand should not wait
            # behind the whole 6.4MB W load.  W is split into slabs in
            # consumption order, alternating between the SP HW queue and the
            # gpsimd software-DGE queue.
            nc.sync.dma_start(xt_sb[:], xt_d[:])
            slabs = [512] * 8
            rest = VSHARD - sum(slabs)
            nrest = 24
            for s in range(nrest):
                slabs.append(rest // nrest + (1 if s < rest % nrest else 0))
            lo = 0
            for s, w in enumerate(slabs):
                eng = nc.sync if s % 2 == 0 else nc.gpsimd
                eng.dma_start(wt_sb[:, lo:lo + w], wt_d[:, lo:lo + w])
                lo += w
            assert lo == VSHARD

            for g in range(B // 128):
                out4_sb = out_pool.tile([128, NW4], BF16, tag="out4")
                out1_sb = out_pool.tile([128, NW1], FP8, tag="out1")
                xg = xt_sb[:, g * 128:(g + 1) * 128]
                i4 = 0
                i1 = 0
                for r in range(NREG + 1):
                    base = r * REG
                    w_cols = REG if r < NREG else TAIL
                    route = ROUTES[r]
                    ps = psum_pool.tile([128, REG], F32)
                    for k in range(0, w_cols, 512):
                        kw = min(512, w_cols - k)
                        nc.tensor.matmul(
                            ps[:, k:k + kw],
                            xg,
                            wt_sb[:, base + k:base + k + kw],
                            start=True, stop=True,
                        )
                    if route == "D":
                        o4 = int(W4_OFF[i4])
                        nc.vector.tensor_reduce(
                            out4_sb[:, o4:o4 + w_cols // 4],
                            ps[:, :w_cols].rearrange("p (n w) -> p n w", w=4),
                            axis=AX, op=MAX,
                        )
                        i4 += 1
                    else:
                        o1 = int(W1_OFF[i1])
                        nc.scalar.activation(
                            out1_sb[:, o1:o1 + w_cols],
                            ps[:, :w_cols],
                            COPY,
                        )
                        i1 += 1
                    for ci, (cr, which, wlo, whi) in enumerate(cuts):
                        if cr != r:
                            continue
                        eng = nc.sync if (g + ci) % 2 == 0 else nc.gpsimd
                        src = out4_sb if which == 4 else out1_sb
                        dst = out4_d if which == 4 else out1_d
                        eng.dma_start(
                            dst[g * 128:(g + 1) * 128, wlo:whi],
                            src[:, wlo:whi],
                        )
    nc.compile()
    return nc


def _build_maps():
    """Per-window candidate columns and EPS.

    Returns (colmap [NWIN, 4] int64 with -1 pads, eps [NWIN] f32) where
    window order is [all w4 windows, all w1 windows] per core.
    """
    nwin = NW4 + NW1
    cm = np.full((nwin, 4), -1, np.int64)
    eps = np.empty(nwin, np.float32)
    for si, (lo, n) in enumerate(W4_SEGS):
        o = int(W4_OFF[si])
        j = np.arange(n // 4)[:, None]
        cm[o:o + n // 4] = lo + 4 * j + np.arange(4)[None, :]
    eps[:NW4] = EPS4
    for si, (lo, n) in enumerate(W1_SEGS):
        o = NW4 + int(W1_OFF[si])
        cm[o:o + n, 0] = lo + np.arange(n)
    eps[NW4:] = EPS1
    return cm, eps


_COLMAP, _WEPS = _build_maps()


def _topk_rows(vals, gidx, k):
    """Per-row top-k ordered like jax.lax.top_k: value desc, index asc."""
    order = np.lexsort((gidx, -vals), axis=-1)[:, :k]
    return (
        np.take_along_axis(gidx, order, axis=1),
        np.take_along_axis(vals, order, axis=1),
    )


def kernel(x: np.ndarray, W: np.ndarray, topk) -> np.ndarray:
    global LAST_RESULTS, _CACHED_NC
    import os

    import ml_dtypes

    from concourse.bass_utils import run_bass_kernel_spmd

    assert x.shape == (B, D) and W.shape == (VOCAB, D)
    assert int(topk) == TOPK
    x = np.ascontiguousarray(np.asarray(x, dtype=np.float32))
    W = np.ascontiguousarray(np.asarray(W, dtype=np.float32))

    if _CACHED_NC is None:
        _CACHED_NC = build_kernel()
    nc = _CACHED_NC

    xt = np.ascontiguousarray(x.T).astype(ml_dtypes.bfloat16)
    in_maps = []
    for i in range(NCORES):
        wt_i = np.ascontiguousarray(
            W[i * VSHARD:(i + 1) * VSHARD].T
        ).astype(ml_dtypes.bfloat16)
        in_maps.append({"wt": wt_i, "xt": xt})

    LAST_RESULTS = run_bass_kernel_spmd(
        nc,
        in_maps,
        core_ids=list(range(NCORES)),
        trace=bool(int(os.environ.get("KERNEL_TRACE", "0"))),
    )
    results = LAST_RESULTS.results

    # [B, 8*(NW4+NW1)] device window values, f32
    nwin = NW4 + NW1
    wm = np.empty((B, NCORES * nwin), np.float32)
    for i in range(NCORES):
        wm[:, i * nwin:i * nwin + NW4] = np.asarray(
            results[i]["out_w4"]).astype(np.float32)
        wm[:, i * nwin + NW4:(i + 1) * nwin] = np.asarray(
            results[i]["out_w1"]).astype(np.float32)
    nwin_all = NCORES * nwin
    weps_all = np.tile(_WEPS, NCORES)

    # Per-row selection on adjusted values v' = v + eps_w:
    # keep windows with v' >= kth_dev - EPSMAX - SLACK.
    wma = wm + weps_all[None, :]
    kth = np.partition(wm, nwin_all - TOPK, axis=1)[:, nwin_all - TOPK]
    tau = kth - EPSMAX - SLACK
    counts = (wma >= tau[:, None]).sum(axis=1)
    K = int(min(max(int(counts.max()), TOPK + 64), 4096))
    topw = np.argpartition(-wma, K - 1, axis=1)[:, :K]  # [B, K] window ids

    core_id = topw // nwin
    wi = topw % nwin
    cols = _COLMAP[wi]  # [B, K, 4], -1 = pad
    pad = cols < 0
    cand = (np.where(pad, 0, cols) + core_id[..., None] * VSHARD).reshape(B, K * 4)

    # Exact f64 re-rank of the candidate columns (pads scored -inf).
    x64 = x.astype(np.float64)
    W64 = W.astype(np.float64)
    exact = np.empty((B, K * 4), np.float64)
    STEP = 64
    for r0 in range(0, B, STEP):
        r1 = r0 + STEP
        gW = W64[cand[r0:r1]]  # [STEP, K*4, D]
        exact[r0:r1] = np.einsum("bjd,bd->bj", gW, x64[r0:r1])
    exact[pad.reshape(B, K * 4)] = -np.inf

    gidx_top, vals_top = _topk_rows(exact, cand, TOPK)

    # Exactness guards: EPS must hold on every selected window (any window
    # that can contain a true top-128 column is selected), and the
    # selection count must fit in K.
    dev_w = np.take_along_axis(wm, topw, axis=1)
    true_w = exact.reshape(B, K, 4).max(axis=2)
    werr = np.abs(dev_w - true_w)
    sel_eps = weps_all[topw]
    err_excess = (werr - sel_eps).max(axis=1)
    bad = (err_excess > 0) | (counts > K)
    if os.environ.get("KERNEL_DEBUG"):
        w4mask = (topw % nwin) < NW4
        e4 = werr[w4mask].max() if w4mask.any() else 0.0
        e1 = werr[~w4mask].max() if (~w4mask).any() else 0.0
        print(f"[kernel] K={K} counts max={counts.max()} "
              f"err4 max={e4:.3f} err1 max={e1:.3f} bad rows={int(bad.sum())}")
    for r in np.flatnonzero(bad):
        s = x64[r] @ W64.T
        gidx_top[r] = np.lexsort((np.arange(VOCAB), -s))[:TOPK]

    return gidx_top.astype(np.int32)
